# revision 7
# baseline (speedup 1.0000x reference)
"""3-layer GAT on 8 Trainium2 cores — v2.

Perf model: the baseline was bound by dma_gather descriptor emission on the
GpSimd Q7 (~8ns/gathered slot, serial). v2:
 - 3 gather streams instead of 5: per-edge adst is selected on-chip via a
   transposed one-hot (PE transpose of the dstoff column + is_equal) and a
   tiny matmul against the local per-group adst table.
 - slot padding 1.195x -> 1.12x: blocks keyed by 256-dst windows; each block
   issues masked lo/hi scatter matmuls (one-hot rows >=128 never match).
 - bf16 tables/gathers/matmuls (fp32 PSUM accumulate).
 - CB=32 blocks (4096 idx) per gather to amortize fixed cost.

Row formats (bf16): l0 [h(256)]=512B ; l1 [h(256)|asrc(4)|pad]=768B ;
l2 [1|h2a|h2b|asrc2|pad]=256B.  e0 (layer-0 attention factors) precomputed
on host from inputs, like the baseline.
"""

import numpy as np

import concourse.bacc as bacc
import concourse.bass as bass
import concourse.mybir as mybir
import concourse.tile as tile
from concourse.bass_utils import run_bass_kernel_spmd

F32 = mybir.dt.float32
BF16 = mybir.dt.bfloat16
I16 = mybir.dt.int16
ALU = mybir.AluOpType
ACTF = mybir.ActivationFunctionType

NEG_SLOPE = 0.2
DEBUG = False


class GATConfig:
    def __init__(self, N, E, DIN, H, C, NCLS, n_cores=8, CB=32):
        self.N, self.E, self.DIN, self.H, self.C, self.NCLS = N, E, DIN, H, C, NCLS
        self.F = H * C
        self.NC = n_cores
        assert N % n_cores == 0
        self.NSH = N // n_cores
        self.NGRP = (self.NSH + 127) // 128
        self.NSHP = self.NGRP * 128
        self.GROWS = self.NSHP * n_cores
        self.HALF = self.GROWS // 2
        assert self.HALF < 32768
        self.SPL = 3200  # core-local row split (25 groups); both gidx
        assert n_cores * self.SPL < 32768  # spaces fit int16
        assert n_cores * (self.NSHP - self.SPL) < 32768
        self.NWIN = (self.NSHP + 255) // 256
        self.CB = CB
        self.EW0 = 256
        self.EW1 = 384
        self.EW2 = 128


def _row_of(cfg, n):
    return (n // cfg.NSH) * cfg.NSHP + (n % cfg.NSH)


def preprocess(cfg, edge_index):
    NC, NSH, CB = cfg.NC, cfg.NSH, cfg.CB
    NWIN = cfg.NGRP  # 128-dst windows == groups
    src = np.asarray(edge_index[0], dtype=np.int64)
    dst = np.asarray(edge_index[1], dtype=np.int64)
    loops = np.arange(cfg.N, dtype=np.int64)
    src = np.concatenate([src, loops])
    dst = np.concatenate([dst, loops])

    core = dst // NSH
    dloc = dst % NSH
    win = dloc // 128
    srow = _row_of(cfg, src)
    SPL = cfg.SPL
    s_local = srow % cfg.NSHP
    s_core = srow // cfg.NSHP
    half = (s_local >= SPL).astype(np.int64)
    gidx = np.where(half == 0, s_core * SPL + s_local,
                    s_core * (cfg.NSHP - SPL) + (s_local - SPL))

    key = (core * 2 + half) * NWIN + win
    counts = np.bincount(key, minlength=NC * 2 * NWIN).reshape(NC, 2, NWIN)
    bpw = np.maximum(1, -(-counts.max(axis=0) // 128))  # [2, NWIN]

    blocks = []  # (half, win, first_in_seg, last_in_seg)
    seg_start = {}
    off = 0
    for p in (0, 1):
        for w in range(NWIN):
            nb = int(bpw[p][w])
            seg_start[(p, w)] = off
            for b in range(nb):
                blocks.append((p, w, b == 0, b == nb - 1))
            off += nb * 128
    nslot = off
    nblk = len(blocks)
    nblk_h = [int(bpw[0].sum()), int(bpw[1].sum())]

    chunks = [[], []]
    for p in (0, 1):
        rem = nblk_h[p]
        while rem > 0:
            take = min(CB, rem)
            chunks[p].append(take)
            rem -= take

    order = np.lexsort((dloc, win, half, core))
    so, do_l, wo, ho, co = (a[order] for a in (src, dloc, win, half, core))
    gi = gidx[order]
    cstart = np.searchsorted(co, np.arange(NC + 1))
    base = np.array([seg_start[(p, w)] for p in (0, 1) for w in range(NWIN)],
                    np.int64)
    per_core = []
    for k in range(NC):
        s0, s1 = cstart[k], cstart[k + 1]
        kh, kw, kd, kgi, ks = ho[s0:s1], wo[s0:s1], do_l[s0:s1], gi[s0:s1], so[s0:s1]
        segkey = kh * NWIN + kw
        starts = np.searchsorted(segkey, np.arange(2 * NWIN))
        rank = np.arange(s1 - s0) - starts[segkey]
        pos = base[segkey] + rank
        g_s = np.zeros(nslot, np.int16)
        f_s = np.full(nslot, -1.0, np.float32)
        sn_s = np.zeros(nslot, np.int32)
        dn_s = np.zeros(nslot, np.int32)
        g_s[pos] = kgi.astype(np.int16)
        f_s[pos] = (kd - kw * 128).astype(np.float32)
        sn_s[pos] = ks.astype(np.int32)
        dn_s[pos] = (k * NSH + kd).astype(np.int32)
        per_core.append((g_s, f_s, sn_s, dn_s))

    meta = dict(blocks=blocks, nblk=nblk, nblk_h=nblk_h, nslot=nslot,
                chunks=chunks)
    return meta, per_core


def _wrap16(a):
    ns = a.size
    w = a.reshape(ns // 16, 16).T
    return np.ascontiguousarray(np.tile(w, (8, 1)))


def _bf(a):
    import ml_dtypes
    return np.asarray(a, dtype=ml_dtypes.bfloat16)


def make_core_inputs(cfg, meta, per_core, xT, weights, e0n):
    asrc0, adst0 = e0n
    nblk = meta["nblk"]
    ins = []
    for k in range(cfg.NC):
        g_s, f_s, sn_s, dn_s = per_core[k]
        xk = np.zeros((cfg.DIN, cfg.NSHP), np.float32)
        xk[:, : cfg.NSH] = xT[:, k * cfg.NSH:(k + 1) * cfg.NSH]
        al = asrc0[sn_s] + adst0[dn_s]
        al = np.where(al >= 0, al, NEG_SLOPE * al)
        e0 = np.exp(al)
        e0[f_s < 0] = 0.0  # pad slots contribute nothing
        # gidx wrapped per chunk, concat along cols -> [128, 8*nblk]
        gcols = []
        boff = 0
        for p in (0, 1):
            for cb in meta["chunks"][p]:
                gcols.append(_wrap16(g_s[boff * 128:(boff + cb) * 128]))
                boff += cb
        m = dict(weights)
        m["xT"] = _bf(xk)
        m["gidxw"] = np.concatenate(gcols, axis=1)
        m["dstoffw"] = _bf(f_s.reshape(nblk, 128).T)          # [128, nblk]
        m["dstor"] = _bf(f_s.reshape(1, -1))                  # [1, nslot]
        m["e0w"] = _bf(e0.reshape(nblk, 128, cfg.H).transpose(1, 0, 2))
        ins.append(m)
    return ins


def make_weights(cfg, W0, a_src0, a_dst0, b0, W1, a_src1, a_dst1, b1,
                 W2, a_src2, a_dst2, b2):
    H, C, F = cfg.H, cfg.C, cfg.F

    def pack(W, a_s, a_d, heads, oc, ncols):
        Wp = np.zeros((W.shape[0], ncols), np.float32)
        Wp[:, : heads * oc] = W
        for h in range(heads):
            Wh = W[:, h * oc:(h + 1) * oc]
            Wp[:, heads * oc + h] = Wh @ a_s[h]
            Wp[:, heads * oc + heads + h] = Wh @ a_d[h]
        return Wp

    w = {
        "W0p": _bf(np.asarray(W0)),                                   # [128,256]
        "W1p": _bf(pack(np.asarray(W1), np.asarray(a_src1),
                        np.asarray(a_dst1), H, C, F + 8)),            # [256,264]
        "W2p": _bf(pack(np.asarray(W2), np.asarray(a_src2),
                        np.asarray(a_dst2), 1, cfg.NCLS, cfg.NCLS + 2)),  # [256,4]
        "b0": np.asarray(b0, np.float32).reshape(1, -1),
        "b1": np.asarray(b1, np.float32).reshape(1, -1),
        "b2": np.asarray(b2, np.float32).reshape(1, -1),
        "iota2": _bf(np.tile(np.arange(256, dtype=np.float32)
                             .reshape(1, 2, 128), (128, 1, 1))),      # [128,2,128]
        "pidx": _bf(np.arange(128, dtype=np.float32).reshape(128, 1)),  # [128,1]
        "pidx2": _bf(np.arange(128, 256, dtype=np.float32).reshape(128, 1)),
        "identb": _bf(np.eye(128, dtype=np.float32)),                 # [128,128]
    }
    return w


def build_kernel(cfg, meta):
    nc = bacc.Bacc("TRN2", target_bir_lowering=False, debug=False,
                   num_devices=cfg.NC)
    F, H, C, NGRP, NWIN, CB = cfg.F, cfg.H, cfg.C, cfg.NGRP, cfg.NWIN, cfg.CB
    blocks = meta["blocks"]
    nblk = meta["nblk"]
    chunks = meta["chunks"]

    xT_d = nc.declare_dram_parameter("xT", [cfg.DIN, cfg.NSHP], BF16, isOutput=False)
    W0p_d = nc.declare_dram_parameter("W0p", [cfg.DIN, F], BF16, isOutput=False)
    W1p_d = nc.declare_dram_parameter("W1p", [F, F + 8], BF16, isOutput=False)
    W2p_d = nc.declare_dram_parameter("W2p", [F, 4], BF16, isOutput=False)
    b0_d = nc.declare_dram_parameter("b0", [1, F], F32, isOutput=False)
    b1_d = nc.declare_dram_parameter("b1", [1, F], F32, isOutput=False)
    b2_d = nc.declare_dram_parameter("b2", [1, cfg.NCLS], F32, isOutput=False)
    iota2_d = nc.declare_dram_parameter("iota2", [128, 2, 128], BF16, isOutput=False)
    pidx_d = nc.declare_dram_parameter("pidx", [128, 1], BF16, isOutput=False)
    pidx2_d = nc.declare_dram_parameter("pidx2", [128, 1], BF16, isOutput=False)
    identb_d = nc.declare_dram_parameter("identb", [128, 128], BF16, isOutput=False)
    gidx_d = nc.declare_dram_parameter("gidxw", [128, 8 * nblk], I16, isOutput=False)
    dsto_d = nc.declare_dram_parameter("dstoffw", [128, nblk], BF16, isOutput=False)
    dstor_d = nc.declare_dram_parameter("dstor", [1, nblk * 128], BF16, isOutput=False)
    e0_d = nc.declare_dram_parameter("e0w", [128, nblk, H], BF16, isOutput=False)
    logit_d = nc.declare_dram_parameter("logits", [cfg.NSHP, cfg.NCLS], F32, isOutput=True)
    prob_d = nc.declare_dram_parameter("probs", [cfg.NSHP, cfg.NCLS], F32, isOutput=True)

    with tile.TileContext(nc) as tc:
        with (
            tc.tile_pool(name="const", bufs=1) as cpool,
            tc.tile_pool(name="acc", bufs=1) as accpool,
            tc.tile_pool(name="lhs", bufs=3) as lhspool,
            tc.tile_pool(name="stage", bufs=3) as stpool,
            tc.tile_pool(name="gath", bufs=3) as gpool,
            tc.tile_pool(name="smat", bufs=2) as spool,
            tc.tile_pool(name="sT", bufs=4) as stT_pool,
            tc.tile_pool(name="msg", bufs=2) as mpool,
            tc.tile_pool(name="meta", bufs=2) as mepool,
            tc.tile_pool(name="alpha", bufs=2) as alpool,
            tc.tile_pool(name="small", bufs=4) as smallpool,
            tc.tile_pool(name="eps", bufs=3, space="PSUM") as epspool,
            tc.tile_pool(name="adps", bufs=2, space="PSUM") as adpspool,
            tc.tile_pool(name="dps", bufs=2, space="PSUM") as dpspool,
            tc.tile_pool(name="dotr", bufs=1, space="PSUM") as dotrpool,
            tc.tile_pool(name="dram", bufs=1, space="DRAM") as drampool,
        ):
            iota2_t = cpool.tile([128, 2, 128], BF16)
            nc.sync.dma_start(iota2_t[:], iota2_d[:])
            pidx_t = cpool.tile([128, 1], BF16)
            nc.sync.dma_start(pidx_t[:], pidx_d[:])
            pidx2_t = cpool.tile([128, 1], BF16, tag="pidx2")
            nc.sync.dma_start(pidx2_t[:], pidx2_d[:])
            ident_t = cpool.tile([128, 128], BF16)
            nc.sync.dma_start(ident_t[:], identb_d[:])

            def load_w(dram, rows, cols, name):
                t = cpool.tile([128, rows // 128, cols], BF16, name=name)
                nc.sync.dma_start(t[:], dram.ap().rearrange("(a p) c -> p a c", p=128))
                return t

            W0p_t = load_w(W0p_d, 128, F, "w0t")
            W1p_t = load_w(W1p_d, 256, F + 8, "w1t")
            W2p_t = load_w(W2p_d, 256, 4, "w2t")
            b0r_t = cpool.tile([128, 1, F], F32, tag="biasb0")
            nc.sync.dma_start(b0r_t[:, 0, :], b0_d.ap().to_broadcast((128, F)))
            b1r_t = cpool.tile([128, 1, F], F32, tag="biasb1")
            nc.sync.dma_start(b1r_t[:, 0, :], b1_d.ap().to_broadcast((128, F)))
            b2r_t = cpool.tile([128, 1, cfg.NCLS], F32, tag="biasb2")
            nc.sync.dma_start(b2r_t[:, 0, :], b2_d.ap().to_broadcast((128, cfg.NCLS)))

            shard0 = drampool.tile([cfg.NSHP, cfg.EW0], BF16)
            shard1 = drampool.tile([cfg.NSHP, cfg.EW1], BF16)
            shard2 = drampool.tile([cfg.NSHP, cfg.EW2], BF16)
            SPL = cfg.SPL
            RA, RB_ = cfg.NC * SPL, cfg.NC * (cfg.NSHP - SPL)
            table0a = drampool.tile([RA, cfg.EW0], BF16, addr_space="Shared")
            table0b = drampool.tile([RB_, cfg.EW0], BF16, addr_space="Shared")
            table1a = drampool.tile([RA, cfg.EW1], BF16, addr_space="Shared")
            table1b = drampool.tile([RB_, cfg.EW1], BF16, addr_space="Shared")
            table2a = drampool.tile([RA, cfg.EW2], BF16, addr_space="Shared")
            table2b = drampool.tile([RB_, cfg.EW2], BF16, addr_space="Shared")

            # bf16 aggregation accumulators [msg | denom] per layer
            accb0 = accpool.tile([128, NGRP, F + H], BF16, tag="accb0")
            accb1 = accpool.tile([128, NGRP, F + H], BF16, tag="accb1")
            acc2 = accpool.tile([128, NGRP, 4], F32, tag="acc2")
            adst1_t = accpool.tile([128, NGRP, H], BF16, tag="adst1")
            adst2_t = accpool.tile([128, NGRP, 1], BF16, tag="adst2")

            # ---------------- dense phases ----------------
            def transpose_to_lhs(src_ap):
                tp = dotrpool.tile([128, 128], BF16, tag="dotr")
                nc.tensor.transpose(out=tp[:], in_=src_ap, identity=ident_t[:])
                lt = lhspool.tile([128, 128], BF16, tag="lhs")
                nc.vector.tensor_copy(lt[:], tp[:])
                return lt

            def dense0(g0=0, g1=None):
                for g in range(g0, NGRP if g1 is None else g1):
                    lt = lhspool.tile([128, 128], BF16, tag="lhs")
                    nc.scalar.dma_start(lt[:], xT_d[:, g * 128:(g + 1) * 128])
                    ps = dpspool.tile([128, F], F32, tag="dps")
                    nc.tensor.matmul(out=ps[:], lhsT=lt[:], rhs=W0p_t[:, 0, :],
                                     start=True, stop=True)
                    st = stpool.tile([128, cfg.EW0], BF16, tag="st0")
                    nc.vector.tensor_copy(st[:], ps[:])
                    nc.scalar.dma_start(shard0[g * 128:(g + 1) * 128, :], st[:])

            def dense1(g0=0, g1=None):
                for g in range(g0, NGRP if g1 is None else g1):
                    ps = dpspool.tile([128, F + 8], F32, tag="dps")
                    for h in range(2):
                        lt = transpose_to_lhs(accb0[:, g, h * 128:(h + 1) * 128])
                        nc.tensor.matmul(out=ps[:], lhsT=lt[:], rhs=W1p_t[:, h, :],
                                         start=(h == 0), stop=(h == 1))
                    st = stpool.tile([128, cfg.EW1], BF16, tag="st1")
                    nc.vector.tensor_copy(st[:, 0:F + 4], ps[:, 0:F + 4])
                    nc.vector.memset(st[:, F + 4:], 0.0)
                    nc.vector.tensor_copy(adst1_t[:, g, :], ps[:, F + 4:F + 8])
                    nc.scalar.dma_start(shard1[g * 128:(g + 1) * 128, :], st[:])

            def dense2(g0=0, g1=None):
                for g in range(g0, NGRP if g1 is None else g1):
                    ps = dpspool.tile([128, 4], F32, tag="dps")
                    for h in range(2):
                        lt = transpose_to_lhs(accb1[:, g, h * 128:(h + 1) * 128])
                        nc.tensor.matmul(out=ps[:], lhsT=lt[:], rhs=W2p_t[:, h, :],
                                         start=(h == 0), stop=(h == 1))
                    st = stpool.tile([128, cfg.EW2], BF16, tag="st2")
                    nc.vector.memset(st[:, 0:1], 1.0)
                    nc.vector.tensor_copy(st[:, 1:4], ps[:, 0:3])
                    nc.vector.memset(st[:, 4:], 0.0)
                    nc.vector.tensor_copy(adst2_t[:, g, :], ps[:, 3:4])
                    nc.scalar.dma_start(shard2[g * 128:(g + 1) * 128, :], st[:])

            def allgather(shard, table, rows):
                nc.gpsimd.collective_compute(
                    "AllGather", ALU.bypass,
                    replica_groups=[list(range(cfg.NC))],
                    ins=[shard[rows[0]:rows[1], :]], outs=[table.opt()],
                )

            # ---------------- edge phase ----------------
            def edge_phase(layer, tabs, ew, accb, adst_loc, nh, msgw,
                           on_done=None):
                edge_psum = {}
                boff = 0
                gc_off = 0
                for p in (0, 1):
                    half = tabs[p][:, :]
                    for cb in chunks[p]:
                        ns = cb * 128
                        gi_t = mepool.tile([128, 8 * CB], I16, tag="gi")
                        nc.sync.dma_start(gi_t[:, :8 * cb],
                                          gidx_d[:, gc_off:gc_off + 8 * cb])
                        do_t = mepool.tile([128, CB], BF16, tag="do")
                        nc.sync.dma_start(do_t[:, :cb], dsto_d[:, boff:boff + cb])

                        g_t = gpool.tile([128, CB, ew], BF16, tag="g")
                        nc.gpsimd.dma_gather(
                            g_t[:, :cb, :], half, gi_t[:, :8 * cb], ns, ns, ew,
                            elem_step=ew, single_packet=False,
                        )
                        # scatter one-hot, batched per chunk
                        slo_t = spool.tile([128, CB, 128], BF16, tag="slo")
                        nc.vector.tensor_tensor(
                            out=slo_t[:, :cb, :],
                            in0=iota2_t[:, 0:1, :].to_broadcast((128, cb, 128)),
                            in1=do_t[:, :cb].to_broadcast((128, cb, 128)),
                            op=ALU.is_equal)

                        al_t = alpool.tile([128, CB, H], BF16, tag="al")
                        if layer == 0:
                            nc.sync.dma_start(al_t[:, :cb, :],
                                              e0_d[:, boff:boff + cb, :])
                        else:
                            # transposed one-hot from partition-replicated
                            # dstoff rows (host data, DRE broadcast DMA)
                            dor_t = spool.tile([128, CB * 128], BF16, tag="dor")
                            nc.sync.dma_start(
                                dor_t[:, :ns],
                                dstor_d[0:1, boff * 128:boff * 128 + ns]
                                .to_broadcast((128, ns)))
                            sT_t = dor_t  # in-place over the dstoff rows
                            nc.vector.tensor_tensor(
                                out=sT_t[:, :ns], in0=dor_t[:, :ns],
                                in1=pidx_t[:].to_broadcast((128, ns)),
                                op=ALU.is_equal)
                            adps = adpspool.tile([128, CB * 4], F32, tag="adps")
                            for b in range(cb):
                                _, w, _, _ = blocks[boff + b]
                                nc.tensor.matmul(
                                    out=adps[:, b * nh:b * nh + nh],
                                    lhsT=sT_t[:, b * 128:(b + 1) * 128],
                                    rhs=adst_loc[:, w, :],
                                    start=True, stop=True)
                            # alpha = asrc + adsel ; lrelu ; exp
                            adb = alpool.tile([128, CB, 4], BF16, tag="adb")
                            nc.vector.tensor_copy(
                                adb[:, :cb, :nh],
                                adps[:, :cb * nh].rearrange(
                                    "p (b h) -> p b h", h=nh))
                            alf = alpool.tile([128, CB, 4], F32, tag="alf")
                            nc.vector.tensor_tensor(
                                out=alf[:, :cb, :nh],
                                in0=g_t[:, :cb, msgw:msgw + nh],
                                in1=adb[:, :cb, :nh],
                                op=ALU.add)
                            nc.scalar.activation(
                                out=alf[:, :cb, :nh], in_=alf[:, :cb, :nh],
                                func=ACTF.Lrelu, alpha=NEG_SLOPE)
                            nc.scalar.activation(
                                out=al_t[:, :cb, :nh], in_=alf[:, :cb, :nh],
                                func=ACTF.Exp)

                        # messages m = [e*h | e]
                        m_t = mpool.tile([128, CB, msgw + 4], BF16, tag="m")
                        nc.vector.tensor_tensor(
                            out=m_t[:, :cb, 0:msgw].rearrange(
                                "p b (h c) -> p b h c", h=nh),
                            in0=g_t[:, :cb, 0:msgw].rearrange(
                                "p b (h c) -> p b h c", h=nh),
                            in1=al_t[:, :cb, :nh]
                                .to_broadcast((128, cb, nh, msgw // nh)),
                            op=ALU.mult)
                        nc.vector.tensor_copy(m_t[:, :cb, msgw:msgw + nh],
                                              al_t[:, :cb, :nh])

                        for b in range(cb):
                            _, w, first, last = blocks[boff + b]
                            if first:
                                pt = epspool.tile([128, msgw + nh], F32, tag="eps")
                                edge_psum[w] = pt
                            pt = edge_psum[w]
                            nc.tensor.matmul(
                                out=pt[:], lhsT=slo_t[:, b, :],
                                rhs=m_t[:, b, 0:msgw + nh],
                                start=first, stop=last)
                            if last:
                                if p == 0:
                                    nc.vector.tensor_copy(accb[:, w, :], pt[:])
                                else:
                                    nc.vector.tensor_tensor(
                                        out=accb[:, w, :], in0=accb[:, w, :],
                                        in1=pt[:], op=ALU.add)
                                    if on_done is not None:
                                        on_done(w)
                        boff += cb
                        gc_off += 8 * cb

            def normalize(accb, bias1, r_t, g0=0, g1=None):
                # batched across a group range; denominators at cols F:F+H.
                # clamp so pad rows can't produce inf/NaN that would leak into
                # the adsel matmul via the adst table
                g1 = NGRP if g1 is None else g1
                ng = g1 - g0
                den = accb[:, g0:g1, F:F + H]
                nc.vector.tensor_scalar_max(out=den, in0=den, scalar1=1e-20)
                nc.vector.reciprocal(r_t[:, g0:g1, :], den)
                nc.vector.tensor_tensor(
                    out=accb[:, g0:g1, 0:F].rearrange(
                        "p g (h c) -> p g h c", h=H),
                    in0=accb[:, g0:g1, 0:F].rearrange(
                        "p g (h c) -> p g h c", h=H),
                    in1=r_t[:, g0:g1, :].to_broadcast((128, ng, H, C)),
                    op=ALU.mult)
                nc.vector.tensor_tensor(
                    out=accb[:, g0:g1, 0:F], in0=accb[:, g0:g1, 0:F],
                    in1=bias1[:].to_broadcast((128, ng, F)),
                    op=ALU.add)
                nc.scalar.activation(out=accb[:, g0:g1, 0:F],
                                     in_=accb[:, g0:g1, 0:F], func=ACTF.Tanh)

            # =========== layers ===========
            r0_t = accpool.tile([128, NGRP, H], F32, tag="r0")
            r1_t = accpool.tile([128, NGRP, H], F32, tag="r1")
            RB = [0, 13, 25, 37, NGRP]  # group-range boundaries
            NG25 = 25  # groups covered by the A row-range [0:SPL]

            def chase(norm_fn, dense_fn, ag_fns):
                def cb(w):
                    for a, b in zip(RB, RB[1:]):
                        if w == b - 1:
                            norm_fn(a, b)
                            dense_fn(a, b)
                    if w in ag_fns:
                        ag_fns[w]()
                return cb

            dense0(0, NG25)
            allgather(shard0, table0a, (0, SPL))
            dense0(NG25, NGRP)
            allgather(shard0, table0b, (SPL, cfg.NSHP))
            edge_phase(0, (table0a, table0b), cfg.EW0, accb0, None, H, F,
                       on_done=chase(
                           lambda a, b: normalize(accb0, b0r_t, r0_t, a, b),
                           dense1,
                           {35: lambda: allgather(shard1, table1a,
                                                   (0, SPL)),
                            NGRP - 1: lambda: allgather(shard1, table1b,
                                                        (SPL, cfg.NSHP))}))
            edge_phase(1, (table1a, table1b), cfg.EW1, accb1, adst1_t, H, F,
                       on_done=chase(
                           lambda a, b: normalize(accb1, b1r_t, r1_t, a, b),
                           dense2,
                           {35: lambda: allgather(shard2, table2a,
                                                   (0, SPL)),
                            NGRP - 1: lambda: allgather(shard2, table2b,
                                                        (SPL, cfg.NSHP))}))
            edge_phase(2, (table2a, table2b), cfg.EW2, acc2, adst2_t, 1, 3)

            # final logits/probs (batched); acc2 rows = [Se, Se*h2a, Se*h2b, Se]
            lg_t = accpool.tile([128, NGRP, cfg.NCLS], F32, tag="lg")
            pb_t = accpool.tile([128, NGRP, cfg.NCLS], F32, tag="pb")
            r2_t = accpool.tile([128, NGRP, 1], F32, tag="r2")
            nc.vector.reciprocal(r2_t[:], acc2[:, :, 0:1])
            nc.vector.tensor_tensor(
                out=lg_t[:], in0=acc2[:, :, 1:3],
                in1=r2_t[:].to_broadcast((128, NGRP, cfg.NCLS)), op=ALU.mult)
            nc.vector.tensor_tensor(
                out=lg_t[:], in0=lg_t[:],
                in1=b2r_t[:].to_broadcast((128, NGRP, cfg.NCLS)), op=ALU.add)
            mx_t = accpool.tile([128, NGRP, 1], F32, tag="mx")
            nc.vector.tensor_tensor(out=mx_t[:], in0=lg_t[:, :, 0:1],
                                    in1=lg_t[:, :, 1:2], op=ALU.max)
            e_t = accpool.tile([128, NGRP, cfg.NCLS], F32, tag="e2")
            nc.vector.tensor_tensor(
                out=e_t[:], in0=lg_t[:],
                in1=mx_t[:].to_broadcast((128, NGRP, cfg.NCLS)),
                op=ALU.subtract)
            nc.scalar.activation(out=e_t[:], in_=e_t[:], func=ACTF.Exp)
            sm_t = accpool.tile([128, NGRP, 1], F32, tag="sm")
            nc.vector.tensor_tensor(out=sm_t[:], in0=e_t[:, :, 0:1],
                                    in1=e_t[:, :, 1:2], op=ALU.add)
            rs_t = accpool.tile([128, NGRP, 1], F32, tag="rs")
            nc.vector.reciprocal(rs_t[:], sm_t[:])
            nc.vector.tensor_tensor(
                out=pb_t[:], in0=e_t[:],
                in1=rs_t[:].to_broadcast((128, NGRP, cfg.NCLS)), op=ALU.mult)
            nc.sync.dma_start(
                logit_d.ap().rearrange("(g p) c -> p g c", p=128), lg_t[:])
            nc.sync.dma_start(
                prob_d.ap().rearrange("(g p) c -> p g c", p=128), pb_t[:])

    nc.compile()
    return nc


# ---------------- public entry point ----------------

_N, _E, _DIN, _H, _C, _NCLS = 50000, 800000, 128, 4, 64, 2


def kernel(x, edge_index, W0, a_src0, a_dst0, b0, W1, a_src1, a_dst1, b1,
           W2, a_src2, a_dst2, b2):
    cfg = GATConfig(_N, _E, _DIN, _H, _C, _NCLS)
    return _run(cfg, x, edge_index, W0, a_src0, a_dst0, b0, W1, a_src1,
                a_dst1, b1, W2, a_src2, a_dst2, b2)


def _run(cfg, x, edge_index, W0, a_src0, a_dst0, b0, W1, a_src1, a_dst1, b1,
         W2, a_src2, a_dst2, b2, trace=False):
    meta, per_core = preprocess(cfg, np.asarray(edge_index))
    weights = make_weights(cfg, W0, a_src0, a_dst0, b0, W1, a_src1, a_dst1,
                           b1, W2, a_src2, a_dst2, b2)
    x = np.asarray(x, np.float32)
    xT = np.ascontiguousarray(x.T)
    h0 = x @ np.asarray(W0)
    asrc0 = (h0.reshape(-1, cfg.H, cfg.C) * np.asarray(a_src0)).sum(-1)
    adst0 = (h0.reshape(-1, cfg.H, cfg.C) * np.asarray(a_dst0)).sum(-1)
    in_maps = make_core_inputs(cfg, meta, per_core, xT, weights,
                               (asrc0, adst0))
    nc = build_kernel(cfg, meta)
    res = run_bass_kernel_spmd(nc, in_maps, list(range(cfg.NC)), trace=trace)
    logits = np.concatenate(
        [res.results[k]["logits"][: cfg.NSH] for k in range(cfg.NC)], axis=0)
    probs = np.concatenate(
        [res.results[k]["probs"][: cfg.NSH] for k in range(cfg.NC)], axis=0)
    if trace:
        kernel.last_exec_time_ns = res.exec_time_ns
        kernel.last_results = res
    return probs, logits
